# revision 7
# baseline (speedup 1.0000x reference)
"""Distributed MultiHeadAttention kernel for 8 TRN2 NeuronCores.

Sharding: core c -> batch b=c//4, head-group g=c%4 (heads 4g..4g+3).
Each core:
  - projects q/k/v for its 4 heads (fp32r matmuls, transposed layouts),
  - computes attention with transposed scores [k,q] so softmax needs no
    attention-matrix transpose (no max subtraction: scores are bounded;
    mask applied multiplicatively in bf16 after exp; softmax denominator
    via a ones-column appended to V),
  - AllToAll exchanges normalized per-head context so each core ends up
    with the full 1024-dim context for its 512-token output slice,
  - O-projection + residual + LayerNorm on that slice.
Host concatenates the 8 output shards.
"""

import sys

for p in ("/opt/trn_rl_repo",):
    if p not in sys.path:
        sys.path.append(p)

import numpy as np
import ml_dtypes

B, S, D, H = 2, 2048, 1024, 16
DK = 64          # head dim
HPC = 4          # heads per core
G = 4            # cores per batch group
TOK = S // G     # 512 output tokens per core
LN_EPS = 1e-5
NCORES = 8

_CACHE = {}


def _build_nc(sim=False):
    import concourse.mybir as mybir
    import concourse.tile as tile
    from concourse import bacc

    f32 = mybir.dt.float32
    f32r = mybir.dt.float32r
    bf16 = mybir.dt.bfloat16
    Exp = mybir.ActivationFunctionType.Exp
    Sqrt = mybir.ActivationFunctionType.Sqrt

    nc = bacc.Bacc("TRN2", target_bir_lowering=False, debug=False, num_devices=1 if sim else NCORES)

    qt = nc.dram_tensor("qt", [D, S], f32r, kind="ExternalInput").ap()        # Q[b].T
    qres = nc.dram_tensor("qres", [TOK, D], f32, kind="ExternalInput").ap()  # Q slice + bo
    maskt = nc.dram_tensor("maskt", [S, S], bf16, kind="ExternalInput").ap() # keep-mask.T
    wq = nc.dram_tensor("wq", [D, HPC * DK], f32r, kind="ExternalInput").ap() # pre-scaled 1/8
    wk = nc.dram_tensor("wk", [D, HPC * DK], f32r, kind="ExternalInput").ap()
    wv = nc.dram_tensor("wv", [D, HPC * DK], f32r, kind="ExternalInput").ap()
    bqk = nc.dram_tensor("bqk", [2 * HPC * DK], f32, kind="ExternalInput").ap()
    bv = nc.dram_tensor("bv", [HPC * DK], f32r, kind="ExternalInput").ap()
    wo = nc.dram_tensor("wo", [2 * D, D], f32r, kind="ExternalInput").ap()  # zero-padded per batch
    gamma = nc.dram_tensor("gamma", [D], f32, kind="ExternalInput").ap()
    beta = nc.dram_tensor("beta", [D], f32, kind="ExternalInput").ap()
    out = nc.dram_tensor("out", [TOK, D], f32, kind="ExternalOutput").ap()

    RG = [[0, 1, 2, 3], [4, 5, 6, 7]]
    NKC = D // 128    # 8 contraction chunks for d_model
    NTC = S // 128    # 16 token chunks
    NQT = S // 512    # 4 query tiles
    VW = HPC * (DK + 1)  # 260: v + per-head ones column

    with tile.TileContext(nc) as tc:
        with (
            tc.tile_pool(name="dram", bufs=1, space="DRAM") as dpool,
            tc.tile_pool(name="consts", bufs=1) as cpool,
            tc.tile_pool(name="qkv", bufs=1) as qkvpool,
        ):
            ctx_local = dpool.tile([2 * G * HPC * DK, TOK], f32r)
            ctx_glob = dpool.tile([2 * G * HPC * DK, TOK], f32r)

            # ---- constants ----
            ones1_f = cpool.tile([1, 128], f32)
            nc.vector.memset(ones1_f[:], 1.0)
            ones1 = cpool.tile([1, 128], f32r)
            nc.vector.tensor_copy(out=ones1[:], in_=ones1_f[:])
            eps_t = cpool.tile([128, 1], f32)
            nc.vector.memset(eps_t[:], LN_EPS)
            bqk_sb = cpool.tile([128, 2, 2], f32)  # [p, proj(q/k), hp]
            nc.sync.dma_start(bqk_sb[:], bqk.rearrange("(w hp p) -> p w hp", p=128, hp=2))
            bv_sb = cpool.tile([1, HPC * DK], f32r)
            nc.sync.dma_start(bv_sb[:], bv.unsqueeze(0))
            grow = cpool.tile([1, D], f32)
            nc.sync.dma_start(grow[:], gamma.unsqueeze(0))
            brow = cpool.tile([1, D], f32)
            nc.sync.dma_start(brow[:], beta.unsqueeze(0))
            gamma_bc = cpool.tile([128, D], f32)
            nc.gpsimd.partition_broadcast(gamma_bc[:], grow[:])
            beta_bc = cpool.tile([128, D], f32)
            nc.gpsimd.partition_broadcast(beta_bc[:], brow[:])

            # ---- persistent qkv activations ----
            qk_sb = qkvpool.tile([128, 2, 2, S], f32r)   # [p, proj(q/k), hp, tokens]
            v_sb = qkvpool.tile([128, NTC, VW], bf16)   # [p, token-chunk, 4*(64+1)]
            v4 = v_sb.rearrange("p t (h x) -> p t h x", x=DK + 1)
            nc.vector.memset(v4[:, :, :, DK : DK + 1], 1.0)

            # ================= Phase 1: projections =================
            with (
                tc.tile_pool(name="qtp", bufs=1) as qtp,
                tc.tile_pool(name="wp", bufs=1) as wp,
                tc.tile_pool(name="pps", bufs=2, space="PSUM") as pps,
                tc.tile_pool(name="ppv", bufs=2, space="PSUM") as ppv,
            ):
                qt_sb = qtp.tile([128, NKC, S], f32r)
                for kc in range(NKC):
                    nc.sync.dma_start(
                        qt_sb[:, kc, :],
                        qt.rearrange("(kc p) s -> p kc s", p=128)[:, kc, :],
                    )
                wq_sb = wp.tile([128, NKC, HPC * DK], f32r)
                wk_sb = wp.tile([128, NKC, HPC * DK], f32r)
                wv_sb = wp.tile([128, NKC, HPC * DK], f32r)
                for w_ap, w_t in ((wq, wq_sb), (wk, wk_sb), (wv, wv_sb)):
                    nc.sync.dma_start(w_t[:], w_ap.rearrange("(kc p) c -> p kc c", p=128))

                # q/k projections -> transposed [dims, tokens] layout
                for proj, w_t in ((0, wq_sb), (1, wk_sb)):
                    for hp in range(2):
                        for nt in range(NQT):
                            ps = pps.tile([128, 512], f32, name="ps_qk", tag="ps_qk")
                            for kc in range(NKC):
                                nc.tensor.matmul(
                                    ps[:],
                                    w_t[:, kc, hp * 128 : (hp + 1) * 128],
                                    qt_sb[:, kc, nt * 512 : (nt + 1) * 512],
                                    start=(kc == 0),
                                    stop=(kc == NKC - 1),
                                )
                            nc.vector.tensor_scalar_add(
                                out=qk_sb[:, proj, hp, nt * 512 : (nt + 1) * 512],
                                in0=ps[:],
                                scalar1=bqk_sb[:, proj, hp : hp + 1],
                            )

                # v projection -> natural [tokens, dims] layout (bf16, ones col)
                for tcn in range(NTC):
                    psv = ppv.tile([128, HPC * DK], f32, name="psv", tag="psv")
                    for kc in range(NKC):
                        nc.tensor.matmul(
                            psv[:],
                            qt_sb[:, kc, tcn * 128 : (tcn + 1) * 128],
                            wv_sb[:, kc, :],
                            start=(kc == 0),
                            stop=False,
                        )
                    nc.tensor.matmul(
                        psv[:],
                        ones1[:, :],
                        bv_sb[:, :],
                        start=False,
                        stop=True,
                    )
                    nc.vector.tensor_copy(
                        out=v4[:, tcn, :, 0:DK],
                        in_=psv.rearrange("p (h x) -> p h x", x=DK),
                    )

            # ================= Phase 2: attention =================
            with (
                tc.tile_pool(name="maskp", bufs=2) as maskp,
                tc.tile_pool(name="ep", bufs=4) as ep,
                tc.tile_pool(name="ap_", bufs=4) as ap_,
                tc.tile_pool(name="sps", bufs=4, space="PSUM") as spsp,
                tc.tile_pool(name="cps", bufs=2, space="PSUM") as cpsp,
                tc.tile_pool(name="nrm", bufs=4) as nrm,
            ):
                for qt_i in range(NQT):
                    mq = maskp.tile([128, NTC, 512], bf16, name="mq", tag="mq")
                    nc.sync.dma_start(
                        mq[:],
                        maskt[:, qt_i * 512 : (qt_i + 1) * 512].rearrange(
                            "(kc p) q -> p kc q", p=128
                        ),
                    )
                    for hp in range(2):
                        cps = [
                            cpsp.tile([DK + 1, 512], f32, name=f"cps{h2}", tag=f"cps{h2}")
                            for h2 in range(2)
                        ]
                        for kc in range(NTC):
                            for h2 in range(2):
                                sp = spsp.tile([128, 512], f32, name="sp", tag="sp")
                                nc.tensor.matmul(
                                    sp[:],
                                    qk_sb[
                                        64 * h2 : 64 * (h2 + 1),
                                        1,
                                        hp,
                                        kc * 128 : (kc + 1) * 128,
                                    ],
                                    qk_sb[
                                        64 * h2 : 64 * (h2 + 1),
                                        0,
                                        hp,
                                        qt_i * 512 : (qt_i + 1) * 512,
                                    ],
                                    start=True,
                                    stop=True,
                                )
                                e = ep.tile([128, 512], bf16, name="e", tag="e")
                                nc.scalar.activation(e[:], sp[:], Exp)
                                a = ap_.tile([128, 512], bf16, name="a", tag="a")
                                nc.vector.tensor_mul(a[:], e[:], mq[:, kc, :])
                                h = 2 * hp + h2
                                nc.tensor.matmul(
                                    cps[h2][:],
                                    v_sb[:, kc, h * (DK + 1) : (h + 1) * (DK + 1)],
                                    a[:],
                                    start=(kc == 0),
                                    stop=(kc == NTC - 1),
                                )
                        for h2 in range(2):
                            h = 2 * hp + h2
                            srec = nrm.tile([1, 512], f32, name="srec", tag="srec")
                            nc.vector.reciprocal(srec[:], cps[h2][DK : DK + 1, :])
                            rbc = nrm.tile([DK, 512], f32, name="rbc", tag="rbc")
                            nc.gpsimd.partition_broadcast(rbc[:], srec[:])
                            ctxn = nrm.tile([DK, 512], f32r, name="ctxn", tag="ctxn")
                            nc.vector.tensor_mul(ctxn[:], cps[h2][0:DK, :], rbc[:])
                            for half in range(2):
                                base = half * (G * HPC * DK) + qt_i * (HPC * DK) + h * DK
                                nc.sync.dma_start(
                                    ctx_local[base : base + DK, :], ctxn[:]
                                )

            # ================= Phase 3: AllToAll =================
            if sim:
                nc.sync.dma_start(ctx_glob[:], ctx_local[:])
            else:
                nc.gpsimd.collective_compute(
                    "AllToAll",
                    mybir.AluOpType.bypass,
                    replica_groups=[list(range(NCORES))],
                    ins=[ctx_local.opt()],
                    outs=[ctx_glob.opt()],
                )

            # ================= Phase 4: O-proj + residual + LN =================
            with (
                tc.tile_pool(name="ctxp", bufs=1) as ctxp,
                tc.tile_pool(name="wop", bufs=1) as wop,
                tc.tile_pool(name="qrp", bufs=1) as qrp,
                tc.tile_pool(name="ops", bufs=4, space="PSUM") as opsp,
                tc.tile_pool(name="oln", bufs=3) as oln,
            ):
                NOC = 2 * D // 128  # 16 contraction chunks (half are zero-Wo)
                ctx_sb = ctxp.tile([128, NOC, TOK], f32r)
                nc.sync.dma_start(
                    ctx_sb[:], ctx_glob.rearrange("(kc p) t -> p kc t", p=128)
                )
                wo_sb = wop.tile([128, NOC, D], f32r)
                nc.sync.dma_start(wo_sb[:], wo.rearrange("(kc p) d -> p kc d", p=128))
                qres_sb = qrp.tile([128, TOK // 128, D], f32)
                nc.sync.dma_start(
                    qres_sb[:], qres.rearrange("(mt p) d -> p mt d", p=128)
                )

                for mt in range(TOK // 128):
                    osb = oln.tile([128, D], f32, name="osb", tag="osb")
                    for nt in range(2):
                        pso = opsp.tile([128, 512], f32, name="pso", tag="pso")
                        for kc in range(NOC):
                            nc.tensor.matmul(
                                pso[:],
                                ctx_sb[:, kc, mt * 128 : (mt + 1) * 128],
                                wo_sb[:, kc, nt * 512 : (nt + 1) * 512],
                                start=(kc == 0),
                                stop=(kc == NOC - 1),
                            )
                        nc.vector.tensor_add(
                            out=osb[:, nt * 512 : (nt + 1) * 512],
                            in0=pso[:],
                            in1=qres_sb[:, mt, nt * 512 : (nt + 1) * 512],
                        )
                    stats = oln.tile([128, 2, 6], f32, name="stats", tag="stats")
                    for sg in range(2):
                        nc.vector.bn_stats(
                            out=stats[:, sg, :], in_=osb[:, sg * 512 : (sg + 1) * 512]
                        )
                    mv = oln.tile([128, 2], f32, name="mv", tag="mv")
                    nc.vector.bn_aggr(out=mv[:], in_=stats[:])
                    rstd = oln.tile([128, 1], f32, name="rstd", tag="rstd")
                    nc.scalar.activation(rstd[:], mv[:, 1:2], Sqrt, bias=eps_t[:])
                    nc.vector.reciprocal(rstd[:], rstd[:])
                    y = oln.tile([128, D], f32, name="y", tag="y")
                    nc.vector.tensor_scalar(
                        out=y[:],
                        in0=osb[:],
                        scalar1=mv[:, 0:1],
                        scalar2=rstd[:],
                        op0=mybir.AluOpType.subtract,
                        op1=mybir.AluOpType.mult,
                    )
                    nc.vector.tensor_mul(y[:], y[:], gamma_bc[:])
                    nc.vector.tensor_add(y[:], y[:], beta_bc[:])
                    nc.sync.dma_start(out[mt * 128 : (mt + 1) * 128, :], y[:])

    nc.compile()
    return nc


def _get_nc():
    if "nc" not in _CACHE:
        _CACHE["nc"] = _build_nc()
    return _CACHE["nc"]


def make_in_maps(inputs):
    Q = np.asarray(inputs["Q"], np.float32)
    mask = np.asarray(inputs["attn_mask"])
    Wq = np.asarray(inputs["Wq"], np.float32)
    Wk = np.asarray(inputs["Wk"], np.float32)
    Wv = np.asarray(inputs["Wv"], np.float32)
    Wo = np.asarray(inputs["Wo"], np.float32)
    bq = np.asarray(inputs["bq"], np.float32)
    bk = np.asarray(inputs["bk"], np.float32)
    bv = np.asarray(inputs["bv"], np.float32)
    bo = np.asarray(inputs["bo"], np.float32)
    gamma = np.asarray(inputs["gamma"], np.float32)
    beta = np.asarray(inputs["beta"], np.float32)
    scale = np.float32(1.0 / np.sqrt(DK))

    in_maps = []
    for c in range(NCORES):
        b, g = c // G, c % G
        hs = slice(g * HPC * DK, (g + 1) * HPC * DK)
        qtb = np.ascontiguousarray(Q[b].T)
        mtb = np.ascontiguousarray((~mask[b]).T).astype(ml_dtypes.bfloat16)
        wo_eff = np.zeros((2 * D, D), np.float32)
        wo_eff[b * D : (b + 1) * D] = Wo
        in_maps.append(
            {
                "qt": qtb,
                "qres": np.ascontiguousarray(Q[b, g * TOK : (g + 1) * TOK]) + bo,
                "maskt": mtb,
                "wq": np.ascontiguousarray(Wq[:, hs]) * scale,
                "wk": np.ascontiguousarray(Wk[:, hs]),
                "wv": np.ascontiguousarray(Wv[:, hs]),
                "bqk": np.concatenate([bq[hs] * scale, bk[hs]]),
                "bv": np.ascontiguousarray(bv[hs]),
                "wo": wo_eff,
                "gamma": gamma,
                "beta": beta,
            }
        )
    return in_maps


def kernel(**inputs):
    from concourse.bass_utils import run_bass_kernel_spmd

    nc = _get_nc()
    in_maps = make_in_maps(inputs)
    res = run_bass_kernel_spmd(nc, in_maps, core_ids=list(range(NCORES)))
    out = np.empty((B, S, D), np.float32)
    for c in range(NCORES):
        b, g = c // G, c % G
        out[b, g * TOK : (g + 1) * TOK] = res.results[c]["out"]
    return out


# revision 9
# speedup vs baseline: 1.0459x; 1.0459x over previous
"""Distributed MultiHeadAttention kernel for 8 TRN2 NeuronCores.

Sharding: core c -> batch b=c//4, head-group g=c%4 (heads 4g..4g+3).
Each core:
  - projects q/k/v for its 4 heads (fp32r matmuls, transposed layouts),
  - computes attention with transposed scores [k,q] so softmax needs no
    attention-matrix transpose (no max subtraction: scores are bounded;
    mask applied multiplicatively in bf16 after exp; softmax denominator
    via a ones-column appended to V),
  - AllToAll exchanges normalized per-head context so each core ends up
    with the full 1024-dim context for its 512-token output slice,
  - O-projection + residual + LayerNorm on that slice.
Host concatenates the 8 output shards.
"""

import sys

for p in ("/opt/trn_rl_repo",):
    if p not in sys.path:
        sys.path.append(p)

import numpy as np
import ml_dtypes

B, S, D, H = 2, 2048, 1024, 16
DK = 64          # head dim
HPC = 4          # heads per core
G = 4            # cores per batch group
TOK = S // G     # 512 output tokens per core
LN_EPS = 1e-5
NCORES = 8

_CACHE = {}


def _build_nc(sim=False):
    import concourse.mybir as mybir
    import concourse.tile as tile
    from concourse import bacc

    f32 = mybir.dt.float32
    f32r = mybir.dt.float32r
    bf16 = mybir.dt.bfloat16
    Exp = mybir.ActivationFunctionType.Exp
    Sqrt = mybir.ActivationFunctionType.Sqrt

    nc = bacc.Bacc("TRN2", target_bir_lowering=False, debug=False, num_devices=1 if sim else NCORES)

    qt = nc.dram_tensor("qt", [D, S], f32r, kind="ExternalInput").ap()        # Q[b].T
    qres = nc.dram_tensor("qres", [TOK, D], f32, kind="ExternalInput").ap()  # Q slice + bo
    maskt = nc.dram_tensor("maskt", [S, S], bf16, kind="ExternalInput").ap() # keep-mask.T
    wq = nc.dram_tensor("wq", [D, HPC * DK], f32r, kind="ExternalInput").ap() # pre-scaled 1/8
    wk = nc.dram_tensor("wk", [D, HPC * DK], f32r, kind="ExternalInput").ap()
    wv = nc.dram_tensor("wv", [D, HPC * DK], f32r, kind="ExternalInput").ap()
    bqk = nc.dram_tensor("bqk", [2 * HPC * DK], f32, kind="ExternalInput").ap()
    bv = nc.dram_tensor("bv", [HPC * DK], f32r, kind="ExternalInput").ap()
    wo = nc.dram_tensor("wo", [2 * D, D], f32r, kind="ExternalInput").ap()  # zero-padded per batch
    gamma = nc.dram_tensor("gamma", [D], f32, kind="ExternalInput").ap()
    beta = nc.dram_tensor("beta", [D], f32, kind="ExternalInput").ap()
    out = nc.dram_tensor("out", [TOK, D], f32, kind="ExternalOutput").ap()

    RG = [[0, 1, 2, 3], [4, 5, 6, 7]]
    NKC = D // 128    # 8 contraction chunks for d_model
    NTC = S // 128    # 16 token chunks
    NQT = S // 512    # 4 query tiles
    VW = HPC * (DK + 1)  # 260: v + per-head ones column

    with tile.TileContext(nc) as tc:
        with (
            tc.tile_pool(name="dram", bufs=1, space="DRAM") as dpool,
            tc.tile_pool(name="consts", bufs=1) as cpool,
            tc.tile_pool(name="qkv", bufs=1) as qkvpool,
        ):
            ctx_local = dpool.tile([2 * G * HPC * DK, TOK], f32r)
            ctx_glob = dpool.tile([2 * G * HPC * DK, TOK], f32r)

            # ---- constants ----
            ones1_f = cpool.tile([1, 128], f32)
            nc.vector.memset(ones1_f[:], 1.0)
            ones1 = cpool.tile([1, 128], f32r)
            nc.vector.tensor_copy(out=ones1[:], in_=ones1_f[:])
            eps_t = cpool.tile([128, 1], f32)
            nc.vector.memset(eps_t[:], LN_EPS)
            bqk_sb = cpool.tile([128, 2, 2], f32)  # [p, proj(q/k), hp]
            nc.sync.dma_start(bqk_sb[:], bqk.rearrange("(w hp p) -> p w hp", p=128, hp=2))
            bv_sb = cpool.tile([1, HPC * DK], f32r)
            nc.sync.dma_start(bv_sb[:], bv.unsqueeze(0))
            grow = cpool.tile([1, D], f32)
            nc.sync.dma_start(grow[:], gamma.unsqueeze(0))
            brow = cpool.tile([1, D], f32)
            nc.sync.dma_start(brow[:], beta.unsqueeze(0))
            gamma_bc = cpool.tile([128, D], f32)
            nc.gpsimd.partition_broadcast(gamma_bc[:], grow[:])
            beta_bc = cpool.tile([128, D], f32)
            nc.gpsimd.partition_broadcast(beta_bc[:], brow[:])

            # ---- persistent qkv activations ----
            qk_sb = qkvpool.tile([128, 2, 2, S], f32r)   # [p, proj(q/k), hp, tokens]
            v_sb = qkvpool.tile([128, NTC, VW], bf16)   # [p, token-chunk, 4*(64+1)]
            v4 = v_sb.rearrange("p t (h x) -> p t h x", x=DK + 1)
            nc.vector.memset(v4[:, :, :, DK : DK + 1], 1.0)

            # ================= Phase 1: projections =================
            with (
                tc.tile_pool(name="qtp", bufs=1) as qtp,
                tc.tile_pool(name="wp", bufs=1) as wp,
                tc.tile_pool(name="pps", bufs=2, space="PSUM") as pps,
                tc.tile_pool(name="ppv", bufs=2, space="PSUM") as ppv,
            ):
                qt_sb = qtp.tile([128, NKC, S], f32r)
                for kc in range(NKC):
                    nc.sync.dma_start(
                        qt_sb[:, kc, :],
                        qt.rearrange("(kc p) s -> p kc s", p=128)[:, kc, :],
                    )
                wq_sb = wp.tile([128, NKC, HPC * DK], f32r)
                wk_sb = wp.tile([128, NKC, HPC * DK], f32r)
                wv_sb = wp.tile([128, NKC, HPC * DK], f32r)
                for w_ap, w_t in ((wq, wq_sb), (wk, wk_sb), (wv, wv_sb)):
                    nc.sync.dma_start(w_t[:], w_ap.rearrange("(kc p) c -> p kc c", p=128))

                # q/k projections -> transposed [dims, tokens] layout
                for proj, w_t in ((0, wq_sb), (1, wk_sb)):
                    for hp in range(2):
                        for nt in range(NQT):
                            ps = pps.tile([128, 512], f32, name="ps_qk", tag="ps_qk")
                            for kc in range(NKC):
                                nc.tensor.matmul(
                                    ps[:],
                                    w_t[:, kc, hp * 128 : (hp + 1) * 128],
                                    qt_sb[:, kc, nt * 512 : (nt + 1) * 512],
                                    start=(kc == 0),
                                    stop=(kc == NKC - 1),
                                )
                            nc.vector.tensor_scalar_add(
                                out=qk_sb[:, proj, hp, nt * 512 : (nt + 1) * 512],
                                in0=ps[:],
                                scalar1=bqk_sb[:, proj, hp : hp + 1],
                            )

                # v projection -> natural [tokens, dims] layout (bf16, ones col)
                for tcn in range(NTC):
                    psv = ppv.tile([128, HPC * DK], f32, name="psv", tag="psv")
                    for kc in range(NKC):
                        nc.tensor.matmul(
                            psv[:],
                            qt_sb[:, kc, tcn * 128 : (tcn + 1) * 128],
                            wv_sb[:, kc, :],
                            start=(kc == 0),
                            stop=False,
                        )
                    nc.tensor.matmul(
                        psv[:],
                        ones1[:, :],
                        bv_sb[:, :],
                        start=False,
                        stop=True,
                    )
                    nc.vector.tensor_copy(
                        out=v4[:, tcn, :, 0:DK],
                        in_=psv.rearrange("p (h x) -> p h x", x=DK),
                    )

            # ================= Phase 2: attention =================
            with (
                tc.tile_pool(name="maskp", bufs=2) as maskp,
                tc.tile_pool(name="ep", bufs=8) as ep,
                tc.tile_pool(name="ap_", bufs=8) as ap_,
                tc.tile_pool(name="sps", bufs=4, space="PSUM") as spsp,
                tc.tile_pool(name="cps", bufs=2, space="PSUM") as cpsp,
                tc.tile_pool(name="nrm", bufs=4) as nrm,
            ):
                for qt_i in range(NQT):
                    mq = maskp.tile([128, NTC, 512], bf16, name="mq", tag="mq")
                    nc.sync.dma_start(
                        mq[:],
                        maskt[:, qt_i * 512 : (qt_i + 1) * 512].rearrange(
                            "(kc p) q -> p kc q", p=128
                        ),
                    )
                    for hp in range(2):
                        cps = [
                            cpsp.tile([DK + 1, 512], f32, name=f"cps{h2}", tag=f"cps{h2}")
                            for h2 in range(2)
                        ]
                        for kc in range(NTC):
                            for h2 in range(2):
                                sp = spsp.tile([128, 512], f32, name="sp", tag="sp")
                                nc.tensor.matmul(
                                    sp[:],
                                    qk_sb[
                                        64 * h2 : 64 * (h2 + 1),
                                        1,
                                        hp,
                                        kc * 128 : (kc + 1) * 128,
                                    ],
                                    qk_sb[
                                        64 * h2 : 64 * (h2 + 1),
                                        0,
                                        hp,
                                        qt_i * 512 : (qt_i + 1) * 512,
                                    ],
                                    start=True,
                                    stop=True,
                                )
                                e = ep.tile([128, 512], bf16, name="e", tag="e")
                                nc.scalar.activation(e[:], sp[:], Exp)
                                a = ap_.tile([128, 512], bf16, name="a", tag="a")
                                nc.vector.tensor_mul(a[:], e[:], mq[:, kc, :])
                                h = 2 * hp + h2
                                nc.tensor.matmul(
                                    cps[h2][:],
                                    v_sb[:, kc, h * (DK + 1) : (h + 1) * (DK + 1)],
                                    a[:],
                                    start=(kc == 0),
                                    stop=(kc == NTC - 1),
                                )
                        for h2 in range(2):
                            h = 2 * hp + h2
                            srec = nrm.tile([1, 512], f32, name="srec", tag="srec")
                            nc.vector.reciprocal(srec[:], cps[h2][DK : DK + 1, :])
                            rbc = nrm.tile([DK, 512], f32, name="rbc", tag="rbc")
                            nc.gpsimd.partition_broadcast(rbc[:], srec[:])
                            ctxn = nrm.tile([DK, 512], f32r, name="ctxn", tag="ctxn")
                            nc.vector.tensor_mul(ctxn[:], cps[h2][0:DK, :], rbc[:])
                            for half in range(2):
                                base = half * (G * HPC * DK) + qt_i * (HPC * DK) + h * DK
                                nc.sync.dma_start(
                                    ctx_local[base : base + DK, :], ctxn[:]
                                )

            # ================= Phase 3: AllToAll =================
            if sim:
                nc.sync.dma_start(ctx_glob[:], ctx_local[:])
            else:
                nc.gpsimd.collective_compute(
                    "AllToAll",
                    mybir.AluOpType.bypass,
                    replica_groups=[list(range(NCORES))],
                    ins=[ctx_local.opt()],
                    outs=[ctx_glob.opt()],
                )

            # ================= Phase 4: O-proj + residual + LN =================
            with (
                tc.tile_pool(name="ctxp", bufs=1) as ctxp,
                tc.tile_pool(name="wop", bufs=1) as wop,
                tc.tile_pool(name="qrp", bufs=1) as qrp,
                tc.tile_pool(name="ops", bufs=4, space="PSUM") as opsp,
                tc.tile_pool(name="oln", bufs=3) as oln,
            ):
                NOC = 2 * D // 128  # 16 contraction chunks (half are zero-Wo)
                ctx_sb = ctxp.tile([128, NOC, TOK], f32r)
                nc.sync.dma_start(
                    ctx_sb[:], ctx_glob.rearrange("(kc p) t -> p kc t", p=128)
                )
                wo_sb = wop.tile([128, NOC, D], f32r)
                nc.sync.dma_start(wo_sb[:], wo.rearrange("(kc p) d -> p kc d", p=128))
                qres_sb = qrp.tile([128, TOK // 128, D], f32)
                nc.sync.dma_start(
                    qres_sb[:], qres.rearrange("(mt p) d -> p mt d", p=128)
                )

                for mt in range(TOK // 128):
                    osb = oln.tile([128, D], f32, name="osb", tag="osb")
                    for nt in range(2):
                        pso = opsp.tile([128, 512], f32, name="pso", tag="pso")
                        for kc in range(NOC):
                            nc.tensor.matmul(
                                pso[:],
                                ctx_sb[:, kc, mt * 128 : (mt + 1) * 128],
                                wo_sb[:, kc, nt * 512 : (nt + 1) * 512],
                                start=(kc == 0),
                                stop=(kc == NOC - 1),
                            )
                        nc.vector.tensor_add(
                            out=osb[:, nt * 512 : (nt + 1) * 512],
                            in0=pso[:],
                            in1=qres_sb[:, mt, nt * 512 : (nt + 1) * 512],
                        )
                    stats = oln.tile([128, 2, 6], f32, name="stats", tag="stats")
                    for sg in range(2):
                        nc.vector.bn_stats(
                            out=stats[:, sg, :], in_=osb[:, sg * 512 : (sg + 1) * 512]
                        )
                    mv = oln.tile([128, 2], f32, name="mv", tag="mv")
                    nc.vector.bn_aggr(out=mv[:], in_=stats[:])
                    rstd = oln.tile([128, 1], f32, name="rstd", tag="rstd")
                    nc.scalar.activation(rstd[:], mv[:, 1:2], Sqrt, bias=eps_t[:])
                    nc.vector.reciprocal(rstd[:], rstd[:])
                    y = oln.tile([128, D], f32, name="y", tag="y")
                    nc.vector.tensor_scalar(
                        out=y[:],
                        in0=osb[:],
                        scalar1=mv[:, 0:1],
                        scalar2=rstd[:],
                        op0=mybir.AluOpType.subtract,
                        op1=mybir.AluOpType.mult,
                    )
                    nc.vector.tensor_mul(y[:], y[:], gamma_bc[:])
                    nc.vector.tensor_add(y[:], y[:], beta_bc[:])
                    nc.sync.dma_start(out[mt * 128 : (mt + 1) * 128, :], y[:])

    nc.compile()
    return nc


def _get_nc():
    if "nc" not in _CACHE:
        _CACHE["nc"] = _build_nc()
    return _CACHE["nc"]


def make_in_maps(inputs):
    Q = np.asarray(inputs["Q"], np.float32)
    mask = np.asarray(inputs["attn_mask"])
    Wq = np.asarray(inputs["Wq"], np.float32)
    Wk = np.asarray(inputs["Wk"], np.float32)
    Wv = np.asarray(inputs["Wv"], np.float32)
    Wo = np.asarray(inputs["Wo"], np.float32)
    bq = np.asarray(inputs["bq"], np.float32)
    bk = np.asarray(inputs["bk"], np.float32)
    bv = np.asarray(inputs["bv"], np.float32)
    bo = np.asarray(inputs["bo"], np.float32)
    gamma = np.asarray(inputs["gamma"], np.float32)
    beta = np.asarray(inputs["beta"], np.float32)
    scale = np.float32(1.0 / np.sqrt(DK))

    in_maps = []
    for c in range(NCORES):
        b, g = c // G, c % G
        hs = slice(g * HPC * DK, (g + 1) * HPC * DK)
        qtb = np.ascontiguousarray(Q[b].T)
        mtb = np.ascontiguousarray((~mask[b]).T).astype(ml_dtypes.bfloat16)
        wo_eff = np.zeros((2 * D, D), np.float32)
        wo_eff[b * D : (b + 1) * D] = Wo
        in_maps.append(
            {
                "qt": qtb,
                "qres": np.ascontiguousarray(Q[b, g * TOK : (g + 1) * TOK]) + bo,
                "maskt": mtb,
                "wq": np.ascontiguousarray(Wq[:, hs]) * scale,
                "wk": np.ascontiguousarray(Wk[:, hs]),
                "wv": np.ascontiguousarray(Wv[:, hs]),
                "bqk": np.concatenate([bq[hs] * scale, bk[hs]]),
                "bv": np.ascontiguousarray(bv[hs]),
                "wo": wo_eff,
                "gamma": gamma,
                "beta": beta,
            }
        )
    return in_maps


def kernel(**inputs):
    from concourse.bass_utils import run_bass_kernel_spmd

    nc = _get_nc()
    in_maps = make_in_maps(inputs)
    res = run_bass_kernel_spmd(nc, in_maps, core_ids=list(range(NCORES)))
    out = np.empty((B, S, D), np.float32)
    for c in range(NCORES):
        b, g = c // G, c % G
        out[b, g * TOK : (g + 1) * TOK] = res.results[c]["out"]
    return out
